# revision 41
# baseline (speedup 1.0000x reference)
"""Trainium2 Bass kernel for a 6-layer GPT-style transformer (ALiBi + causal),
data-parallel over batch across 8 NeuronCores (1 sequence per core).

v2 layout strategy per core (N=1024 tokens, E=768, H=12, DH=64):
  - Residual x: token-major SBUF [128, 8, 768] fp32 (partition = token % 128)
  - LN on token-major tiles; normalized y cast to bf16, PE-transposed to
    feature-major yT [128, 6, 1024] bf16.
  - Every GEMM uses bf16 stationary+moving (fp32 PSUM accumulation):
    cheap LDWEIGHTS and 1 cycle/row streaming.
  - q/k projections: per-head chains -> psum [64, 512], copied into
    persistent 65-partition tiles q65/k65 [65, 12, 1024] bf16 whose row 64
    holds constants written once: q row 64 = -slope_h*i/SCALE (safe-exp
    shift per query), k row 64 = 1.0.  The score matmul then computes
    k65^T @ q65 with a 65-deep contraction: scores + per-query shift in a
    SINGLE matmul per 512-wide chunk (no rank-1 bias matmul).
  - ALiBi "+slope*j" via per-partition ACT bias at the exp; causal mask via
    DVE min with 4 precomputed [128,512] triangle tiles (one per kb%4).
  - attn*v and the softmax denominator are two chains into one [128,512]
    psum pair: even head -> partitions 0:64, odd head -> 64:128 (PE
    tile_position), so normalized oT lands pre-paired [128, 6, 1024] and
    out-proj contracts 128-deep in 6 steps.
  - MLP: fc1 feature-major (gelu fused via ACT), fc2 token-major, both bf16.
"""

import math
import os
import sys

import numpy as np

sys.path.insert(0, "/opt/trn_rl_repo")

import concourse.bass as bass  # noqa: E402
import concourse.mybir as mybir  # noqa: E402
import concourse.tile as tile  # noqa: E402
from concourse import bacc  # noqa: E402

P = 128
B, N, E, H, DEPTH, A = 8, 1024, 768, 12, 6, 7
DH = E // H  # 64
F = 4 * E  # 3072
NT = N // P  # 8 token tiles
EC = E // P  # 6 E chunks
FC = F // P  # 24 F chunks
QC = 512  # query chunk for scores / o
SCALE = DH ** -0.5  # 0.125
EPS = 1e-6
NEG = -1.0e30
BIG = 3.0e38

f32 = mybir.dt.float32
bf16 = mybir.dt.bfloat16
AF = mybir.ActivationFunctionType
ALU = mybir.AluOpType

# Heads are processed in packed pairs (slotA, slotB). Slot A (partitions
# 0..64) carries the exact per-query safe-exp shift via a 65th contraction
# row; slot B heads have small ALiBi slopes, so a constant re-centered exp
# bias slope*(j-512) is overflow-safe and softmax-invariant — no shift
# needed. PERM packs the 5 big-slope heads (+h2) into slot A.
PERM = [0, 3, 1, 4, 8, 5, 9, 6, 10, 7, 2, 11]
J0 = 512  # re-center offset for slot-B exp bias

# att strip kb stores query cols [qstart(kb), N), qstart = (kb//4)*512
ATT_OFF = [0, 1024, 2048, 3072, 4096, 4608, 5120, 5632]
ATT_TOT = 6144


def _slopes(n):
    def p2(n):
        start = 2 ** (-(2 ** -(math.log2(n) - 3)))
        return [start * start**i for i in range(n)]

    if math.log2(n).is_integer():
        return p2(n)
    c = 2 ** math.floor(math.log2(n))
    return p2(c) + _slopes(2 * c)[0::2][: n - c]


def build_program(sim_gelu=False, finalize=True, depth=DEPTH,
                  skip_attn=False, skip_mlp=False):
    nc = bacc.Bacc()

    dp = nc.declare_dram_parameter
    x_d = dp("x", [N, E], f32, isOutput=False)
    wqkv_d = dp("wqkv", [DEPTH, E, 3 * E], bf16, isOutput=False)
    wo_d = dp("wo_t", [DEPTH, E, E], bf16, isOutput=False)
    w1_d = dp("w1", [DEPTH, E, F], bf16, isOutput=False)
    b1_d = dp("b1_t", [DEPTH, P, FC], f32, isOutput=False)
    w2_d = dp("w2_t", [DEPTH, F, E], bf16, isOutput=False)
    hw1_d = dp("hw1", [E, E], bf16, isOutput=False)
    hb1_d = dp("hb1_t", [P, EC], f32, isOutput=False)
    hw2_d = dp("hw2", [E, 8], bf16, isOutput=False)  # A=7 padded to 8
    aqe_d = dp("alibi_qe", [EC, N], bf16, isOutput=False)  # slot-A shift rows
    ko_d = dp("kones", [EC, N], bf16, isOutput=False)  # ones row for slot-A k65
    ab_d = dp("alibi_b", [P, H * NT], f32, isOutput=False)  # slope_h*(128kb+p)
    mk_d = dp("masks", [P, 4, QC], f32, isOutput=False)  # causal masks by kb%4
    id_d = dp("ident", [P, P], bf16, isOutput=False)
    out_d = dp("out", [N, A], f32, isOutput=True)

    with tile.TileContext(nc) as tc:
        with tc.tile_pool(name="persist", bufs=1) as pp:
            x_sb = pp.tile([P, NT, E], f32)
            yT = pp.tile([P, EC, N], bf16)
            oT = pp.tile([P, EC, N], bf16)
            ones64 = pp.tile([P, DH], bf16)
            ab_sb = pp.tile([P, H * NT], f32)
            mk_sb = pp.tile([P, 4, QC], f32)
            id_sb = pp.tile([P, P], bf16)

            nc.sync.dma_start(x_sb[:], x_d.rearrange("(t p) e -> p t e", p=P))
            nc.sync.dma_start(ab_sb[:], ab_d[:])
            nc.sync.dma_start(mk_sb[:], mk_d[:])
            nc.sync.dma_start(id_sb[:], id_d[:])
            nc.vector.memset(ones64[:], 1.0)

            czero = pp.tile([P, 1], f32)
            ceps = pp.tile([P, 1], f32)
            nc.vector.memset(czero[:], 0.0)
            nc.vector.memset(ceps[:], EPS)
            nc.const_aps.aps[(f32, 0.0)] = czero[:]
            nc.const_aps.aps[(f32, EPS)] = ceps[:]

            for layer in range(depth):
                _layernorm_to_yT(nc, tc, x_sb, yT, id_sb)
                if not skip_attn:
                    _attn_block(nc, tc, layer, x_sb, yT, oT, ones64,
                                ab_sb, mk_sb, wqkv_d, wo_d, aqe_d, ko_d)
                    _layernorm_to_yT(nc, tc, x_sb, yT, id_sb)
                if not skip_mlp:
                    _mlp_block(nc, tc, layer, x_sb, yT, w1_d, b1_d, w2_d,
                               sim_gelu)

            _head(nc, tc, x_sb, yT, id_sb, hw1_d, hb1_d, hw2_d, out_d)

    if finalize:
        nc.finalize()
    return nc


def _layernorm_to_yT(nc, tc, x_sb, yT, id_sb):
    """token-major LN over x_sb -> feature-major bf16 yT (PE transpose)."""
    from contextlib import ExitStack

    with ExitStack() as ctx:
        lp = ctx.enter_context(tc.tile_pool(name="ln", bufs=2))
        sp = ctx.enter_context(tc.tile_pool(name="lns", bufs=1))
        ps = ctx.enter_context(tc.tile_pool(name="lnp", bufs=2, space="PSUM"))

        stats = sp.tile([P, NT, 2], f32)
        rstd = sp.tile([P, NT], f32)
        nmr = sp.tile([P, NT], f32)
        for half in range(2):
            t0, t1 = half * 4, half * 4 + 4
            for t in range(t0, t1):
                st6 = lp.tile([P, 2, 6], f32, tag="st6")
                nc.vector.bn_stats(st6[:, 0, :], x_sb[:, t, 0:384])
                nc.vector.bn_stats(st6[:, 1, :], x_sb[:, t, 384:768])
                nc.vector.bn_aggr(stats[:, t, :], st6[:])
            sl = slice(t0, t1)
            nc.scalar.activation(stats[:, sl, 1], stats[:, sl, 1], AF.Ln, bias=EPS)
            nc.scalar.activation(rstd[:, sl], stats[:, sl, 1], AF.Exp, scale=-0.5)
            nc.vector.tensor_tensor(nmr[:, sl], stats[:, sl, 0], rstd[:, sl], ALU.mult)
            nc.vector.tensor_scalar_mul(nmr[:, sl], nmr[:, sl], -1.0)
            for t in range(t0, t1):
                y_t = lp.tile([P, E], bf16, tag="y")
                nc.scalar.activation(
                    y_t[:], x_sb[:, t, :], AF.Identity,
                    bias=nmr[:, t : t + 1], scale=rstd[:, t : t + 1],
                )
                tp = ps.tile([P, E], bf16, tag="tp")
                for c in range(EC):
                    nc.tensor.transpose(tp[:, c * P : (c + 1) * P], y_t[:, c * P : (c + 1) * P], id_sb[:])
                nc.vector.tensor_copy(
                    yT[:, :, t * P : (t + 1) * P],
                    tp[:].rearrange("p (c q) -> p c q", q=P),
                )


def _attn_block(nc, tc, layer, x_sb, yT, oT, ones64,
                ab_sb, mk_sb, wqkv_d, wo_d, aqe_d, ko_d):
    from contextlib import ExitStack

    with ExitStack() as octx:
        op = octx.enter_context(tc.tile_pool(name="qk5", bufs=1))
        # slot-A heads: 65-row tiles at partitions 0..64 (row 64 = constants)
        q65 = op.tile([DH + 1, EC, N], bf16)
        k65 = op.tile([DH + 1, EC, N], bf16)
        # slot-B heads: rows 64..127 (constant re-centered bias, no shift)
        qo = op.tile([P, EC, N], bf16)
        ko = op.tile([P, EC, N], bf16)
        vhat = op.tile([P, NT, E], bf16)
        nc.sync.dma_start(q65[DH : DH + 1, :, :], aqe_d.rearrange("(o g) n -> o g n", o=1))
        nc.sync.dma_start(k65[DH : DH + 1, :, :], ko_d.rearrange("(o g) n -> o g n", o=1))
        _attn_inner(nc, tc, layer, x_sb, yT, q65, k65, qo, ko, oT, vhat,
                    ones64, ab_sb, mk_sb, wqkv_d, wo_d)


def _attn_inner(nc, tc, layer, x_sb, yT, q65, k65, qo, ko, oT, vhat,
                ones64, ab_sb, mk_sb, wqkv_d, wo_d):
    from contextlib import ExitStack

    # ---------- v = y @ wv  (token-major) ----------
    with ExitStack() as ctx:
        wp = ctx.enter_context(tc.tile_pool(name="vw", bufs=2))
        ps = ctx.enter_context(tc.tile_pool(name="vp", bufs=2, space="PSUM"))
        vw0 = wp.tile([P, EC, 512], bf16, tag="vw0")
        vw1 = wp.tile([P, EC, 256], bf16, tag="vw1")
        nc.sync.dma_start(
            vw0[:], wqkv_d[layer, :, 2 * E : 2 * E + 512].rearrange("(o p) c -> p o c", p=P)
        )
        nc.sync.dma_start(
            vw1[:], wqkv_d[layer, :, 2 * E + 512 :].rearrange("(o p) c -> p o c", p=P)
        )
        for t in range(NT):
            pv0 = ps.tile([P, 512], f32, tag="pv0")
            pv1 = ps.tile([P, 256], f32, tag="pv1")
            for ec in range(EC):
                nc.tensor.matmul(
                    pv0[:], yT[:, ec, t * P : (t + 1) * P], vw0[:, ec, :],
                    start=(ec == 0), stop=(ec == EC - 1),
                )
            for ec in range(EC):
                nc.tensor.matmul(
                    pv1[:], yT[:, ec, t * P : (t + 1) * P], vw1[:, ec, :],
                    start=(ec == 0), stop=(ec == EC - 1),
                )
            nc.vector.tensor_copy(vhat[:, t, 0:512], pv0[:])
            nc.vector.tensor_copy(vhat[:, t, 512:768], pv1[:])

    # ---------- q/k projections: one pair of heads per [128,512] psum ----------
    with ExitStack() as ctx:
        qkw = ctx.enter_context(tc.tile_pool(name="qkw", bufs=3))
        pqk = ctx.enter_context(tc.tile_pool(name="pqk", bufs=3, space="PSUM"))
        for g in range(H // 2):
            qw = qkw.tile([P, EC, P], bf16, tag="qw")
            kw = qkw.tile([P, EC, P], bf16, tag="kw")
            nc.sync.dma_start(
                qw[:], wqkv_d[layer, :, g * P : (g + 1) * P].rearrange("(o p) c -> p o c", p=P)
            )
            nc.sync.dma_start(
                kw[:], wqkv_d[layer, :, E + g * P : E + (g + 1) * P].rearrange("(o p) c -> p o c", p=P)
            )
            for tb in range(2):
                tsl = slice(tb * 512, (tb + 1) * 512)
                pq = pqk.tile([P, 512], f32, tag="pqk")
                pk = pqk.tile([P, 512], f32, tag="pqk")
                for ec in range(EC):
                    nc.tensor.matmul(
                        pq[:], qw[:, ec, :], yT[:, ec, tsl],
                        start=(ec == 0), stop=(ec == EC - 1),
                    )
                for ec in range(EC):
                    nc.tensor.matmul(
                        pk[:], kw[:, ec, :], yT[:, ec, tsl],
                        start=(ec == 0), stop=(ec == EC - 1),
                    )
                nc.scalar.copy(q65[0:DH, g, tsl], pq[0:DH, :])
                nc.scalar.copy(qo[DH:P, g, tsl], pq[DH:P, :])
                nc.vector.tensor_copy(k65[0:DH, g, tsl], pk[0:DH, :])
                nc.vector.tensor_copy(ko[DH:P, g, tsl], pk[DH:P, :])

    # ---------- scores/exp + o/r ----------
    with ExitStack() as ctx:
        att = ctx.enter_context(tc.tile_pool(name="att", bufs=4))
        rcp = ctx.enter_context(tc.tile_pool(name="rcp", bufs=2))
        pst = ctx.enter_context(tc.tile_pool(name="pst", bufs=4, space="PSUM"))
        pov = ctx.enter_context(tc.tile_pool(name="pov", bufs=2, space="PSUM"))
        prv = ctx.enter_context(tc.tile_pool(name="prv", bufs=2, space="PSUM"))

        att_h = {}

        def emit_or(g):
            # o / r for pair g (issued one pair late so its exps are drained)
            for qb in range(2):
                po = pov.tile([P, QC], f32, tag="po")
                pr = prv.tile([P, QC], f32, tag="pr")
                kbs = list(range(4 * (qb + 1)))
                for hh in range(2):
                    hcur = 2 * g + hh
                    a = att_h[hcur]
                    osl = slice(hh * DH, (hh + 1) * DH)
                    for kb in kbs:
                        off = ATT_OFF[kb] + qb * 512 - (kb // 4) * 512
                        nc.tensor.matmul(
                            po[osl, :], vhat[:, kb, hcur * DH : (hcur + 1) * DH],
                            a[:, off : off + QC],
                            start=(kb == 0), stop=(kb == kbs[-1]),
                        )
                    for kb in kbs:
                        off = ATT_OFF[kb] + qb * 512 - (kb // 4) * 512
                        nc.tensor.matmul(
                            pr[osl, :], ones64[:], a[:, off : off + QC],
                            start=(kb == 0), stop=(kb == kbs[-1]),
                        )
                rec = rcp.tile([P, QC], f32, tag="rec")
                nc.vector.reciprocal_approx_fast(rec[:], pr[:])
                nc.vector.tensor_tensor(
                    oT[:, g, qb * QC : (qb + 1) * QC], po[:], rec[:], ALU.mult
                )

        for h in range(H):
            g = h // 2
            # -- scores + exp for head h --
            at = att.tile([P, ATT_TOT], bf16, tag="at")
            att_h[h] = at
            for kb in range(NT):
                qstart = (kb // 4) * 512
                m4 = kb % 4
                for ci, q0 in enumerate(range(qstart, N, QC)):
                    st = pst.tile([P, QC], f32, tag="st")
                    if h % 2 == 0:
                        nc.tensor.matmul(
                            st[:],
                            k65[:, g, kb * P : (kb + 1) * P],
                            q65[:, g, q0 : q0 + QC],
                            start=True, stop=True,
                        )
                    else:
                        nc.tensor.matmul(
                            st[:],
                            ko[DH:P, g, kb * P : (kb + 1) * P],
                            qo[DH:P, g, q0 : q0 + QC],
                            start=True, stop=True,
                        )
                    if ci == 0:
                        w = (m4 + 1) * P
                        nc.vector.tensor_tensor(
                            st[:, 0:w], st[:, 0:w], mk_sb[:, m4, 0:w], ALU.min
                        )
                    nc.scalar.activation(
                        at[:, ATT_OFF[kb] + q0 - qstart : ATT_OFF[kb] + q0 - qstart + QC],
                        st[:], AF.Exp,
                        bias=ab_sb[:, h * NT + kb : h * NT + kb + 1], scale=SCALE,
                    )
            if h % 2 == 1 and g >= 1:
                emit_or(g - 1)
        emit_or(H // 2 - 1)

    # ---------- out-proj + residual ----------
    with ExitStack() as ctx:
        wp = ctx.enter_context(tc.tile_pool(name="wop", bufs=1))
        ps = ctx.enter_context(tc.tile_pool(name="pop", bufs=2, space="PSUM"))
        wo_sb = wp.tile([P, EC, E], bf16)
        nc.sync.dma_start(wo_sb[:], wo_d[layer].rearrange("(o p) c -> p o c", p=P))
        for t in range(NT):
            po0 = ps.tile([P, 512], f32, tag="po0")
            po1 = ps.tile([P, 256], f32, tag="po1")
            for g in range(EC):
                nc.tensor.matmul(
                    po0[:], oT[:, g, t * P : (t + 1) * P], wo_sb[:, g, 0:512],
                    start=(g == 0), stop=(g == EC - 1),
                )
            for g in range(EC):
                nc.tensor.matmul(
                    po1[:], oT[:, g, t * P : (t + 1) * P], wo_sb[:, g, 512:768],
                    start=(g == 0), stop=(g == EC - 1),
                )
            nc.vector.tensor_tensor(x_sb[:, t, 0:512], po0[:], x_sb[:, t, 0:512], ALU.add)
            nc.vector.tensor_tensor(x_sb[:, t, 512:768], po1[:], x_sb[:, t, 512:768], ALU.add)


def _mlp_block(nc, tc, layer, x_sb, yT, w1_d, b1_d, w2_d, sim_gelu):
    from contextlib import ExitStack

    with ExitStack() as ctx:
        wp = ctx.enter_context(tc.tile_pool(name="mw", bufs=4))
        hp = ctx.enter_context(tc.tile_pool(name="hT", bufs=1))
        bp = ctx.enter_context(tc.tile_pool(name="b1", bufs=1))
        ps1 = ctx.enter_context(tc.tile_pool(name="pf1", bufs=2, space="PSUM"))
        ps2 = ctx.enter_context(tc.tile_pool(name="pf2", bufs=2, space="PSUM"))

        hT = hp.tile([P, FC, N], bf16)
        b1 = bp.tile([P, FC], f32)
        nc.sync.dma_start(b1[:], b1_d[layer])
        for fc in range(FC):
            w1c = wp.tile([P, EC, P], bf16, tag="w1c")
            nc.sync.dma_start(
                w1c[:], w1_d[layer, :, fc * P : (fc + 1) * P].rearrange("(o p) c -> p o c", p=P)
            )
            for tb in range(2):
                pf = ps1.tile([P, 512], f32, tag="pf")
                for ec in range(EC):
                    nc.tensor.matmul(
                        pf[:], w1c[:, ec, :], yT[:, ec, tb * 512 : (tb + 1) * 512],
                        start=(ec == 0), stop=(ec == EC - 1),
                    )
                dst = hT[:, fc, tb * 512 : (tb + 1) * 512]
                if not sim_gelu:
                    nc.scalar.activation(
                        dst, pf[:], AF.Gelu_apprx_tanh, bias=b1[:, fc : fc + 1]
                    )
                else:
                    u = wp.tile([P, 512], f32, tag="sg_u")
                    s = wp.tile([P, 512], f32, tag="sg_s")
                    th = wp.tile([P, 512], f32, tag="sg_t")
                    nc.scalar.activation(u[:], pf[:], AF.Identity, bias=b1[:, fc : fc + 1])
                    nc.vector.tensor_tensor(s[:], u[:], u[:], ALU.mult)
                    nc.vector.tensor_scalar(s[:], s[:], 0.044715, 1.0, ALU.mult, ALU.add)
                    nc.vector.tensor_tensor(s[:], s[:], u[:], ALU.mult)
                    nc.scalar.activation(th[:], s[:], AF.Tanh, scale=0.7978845608028654)
                    nc.vector.tensor_tensor(th[:], th[:], u[:], ALU.mult)
                    nc.vector.tensor_tensor(th[:], th[:], u[:], ALU.add)
                    nc.vector.tensor_scalar_mul(dst, th[:], 0.5)
        w2_sb = hp.tile([P, FC, E], bf16)
        nc.sync.dma_start(w2_sb[:], w2_d[layer].rearrange("(o p) c -> p o c", p=P))
        for t in range(NT):
            pm0 = ps2.tile([P, 512], f32, tag="pm0")
            pm1 = ps2.tile([P, 256], f32, tag="pm1")
            for fc in range(FC):
                nc.tensor.matmul(
                    pm0[:], hT[:, fc, t * P : (t + 1) * P], w2_sb[:, fc, 0:512],
                    start=(fc == 0), stop=(fc == FC - 1),
                )
            for fc in range(FC):
                nc.tensor.matmul(
                    pm1[:], hT[:, fc, t * P : (t + 1) * P], w2_sb[:, fc, 512:768],
                    start=(fc == 0), stop=(fc == FC - 1),
                )
            nc.vector.tensor_tensor(x_sb[:, t, 0:512], pm0[:], x_sb[:, t, 0:512], ALU.add)
            nc.vector.tensor_tensor(x_sb[:, t, 512:768], pm1[:], x_sb[:, t, 512:768], ALU.add)


def _head(nc, tc, x_sb, yT, id_sb, hw1_d, hb1_d, hw2_d, out_d):
    from contextlib import ExitStack

    _layernorm_to_yT(nc, tc, x_sb, yT, id_sb)
    with ExitStack() as ctx:
        wp = ctx.enter_context(tc.tile_pool(name="hw", bufs=1))
        hp = ctx.enter_context(tc.tile_pool(name="h1", bufs=1))
        op = ctx.enter_context(tc.tile_pool(name="ot", bufs=1))
        ps = ctx.enter_context(tc.tile_pool(name="ph", bufs=2, space="PSUM"))

        hb1 = wp.tile([P, EC], f32)
        nc.sync.dma_start(hb1[:], hb1_d[:])
        hw2 = wp.tile([P, EC, 8], bf16)
        nc.sync.dma_start(hw2[:], hw2_d.rearrange("(o p) a -> p o a", p=P))
        h1T = hp.tile([P, EC, N], bf16)
        for oc in range(EC):
            w1c = wp.tile([P, EC, P], bf16, tag="hw1c", bufs=2)
            nc.sync.dma_start(
                w1c[:], hw1_d[:, oc * P : (oc + 1) * P].rearrange("(o p) c -> p o c", p=P)
            )
            for tb in range(2):
                pf = ps.tile([P, 512], f32, tag="pf")
                for ec in range(EC):
                    nc.tensor.matmul(
                        pf[:], w1c[:, ec, :], yT[:, ec, tb * 512 : (tb + 1) * 512],
                        start=(ec == 0), stop=(ec == EC - 1),
                    )
                nc.scalar.activation(
                    h1T[:, oc, tb * 512 : (tb + 1) * 512], pf[:], AF.Relu,
                    bias=hb1[:, oc : oc + 1],
                )
        out_sb = op.tile([P, NT, A], f32)
        for t in range(NT):
            pa = ps.tile([P, 8], f32, tag="pa")
            for ec in range(EC):
                nc.tensor.matmul(
                    pa[:], h1T[:, ec, t * P : (t + 1) * P], hw2[:, ec, :],
                    start=(ec == 0), stop=(ec == EC - 1),
                )
            nc.vector.tensor_copy(out_sb[:, t, :], pa[:, 0:A])
        nc.sync.dma_start(out_d.rearrange("(t p) a -> p t a", p=P), out_sb[:])


# ---------------------------------------------------------------- host side

_CACHE = {}


def ml_bf16():
    import ml_dtypes

    return ml_dtypes.bfloat16


def _host_prep(inputs):
    bf = ml_bf16()
    slopes = np.asarray(_slopes(H), np.float32)
    i_idx = np.arange(N, dtype=np.float32)

    ln1_s, ln1_b = np.asarray(inputs["ln1_scale"]), np.asarray(inputs["ln1_bias"])
    ln2_s, ln2_b = np.asarray(inputs["ln2_scale"]), np.asarray(inputs["ln2_bias"])
    lnf_s, lnf_b = np.asarray(inputs["lnf_scale"]), np.asarray(inputs["lnf_bias"])
    wqkv, bqkv = np.asarray(inputs["wqkv"]), np.asarray(inputs["bqkv"])
    wo, bo = np.asarray(inputs["wo"]), np.asarray(inputs["bo"])
    w1, w2 = np.asarray(inputs["w1"]), np.asarray(inputs["w2"])
    hw1, hb1 = np.asarray(inputs["head_w1"]), np.asarray(inputs["head_b1"])
    hw2 = np.asarray(inputs["head_w2"])

    # fold LN affine into following matmuls (exact algebra)
    wqkv_eff = ln1_s[:, :, None] * wqkv
    bqkv_eff = bqkv + np.einsum("le,lec->lc", ln1_b, wqkv)
    w1_eff = ln2_s[:, :, None] * w1
    b1_eff = np.einsum("le,lef->lf", ln2_b, w1)
    hw1_eff = lnf_s[:, None] * hw1
    hb1_eff = hb1 + lnf_b @ hw1

    # pack heads in PERM order: q/k/v weight columns and wo rows
    hidx = np.concatenate([np.arange(p * DH, (p + 1) * DH) for p in PERM])
    wqkv_eff = np.concatenate(
        [wqkv_eff[:, :, hidx], wqkv_eff[:, :, E + hidx], wqkv_eff[:, :, 2 * E + hidx]],
        axis=2,
    )
    wo_eff = wo[:, hidx, :]

    b1_t = np.ascontiguousarray(b1_eff.reshape(DEPTH, FC, P).transpose(0, 2, 1))
    hb1_t = np.ascontiguousarray(hb1_eff.reshape(EC, P).T)

    aq_row = ((-slopes[:, None] / SCALE) * i_idx[None, :]).astype(bf)  # [H,N]
    alibi_qe = np.ascontiguousarray(aq_row[PERM[0::2]])  # slot-A heads
    alibi_b = np.zeros((P, H * NT), np.float32)
    p_idx = np.arange(P, dtype=np.float32)
    for s in range(H):
        t = PERM[s]
        j0 = 0 if s % 2 == 0 else J0  # slot-B: overflow-safe re-centering
        for kb in range(NT):
            alibi_b[:, s * NT + kb] = slopes[t] * (P * kb + p_idx - j0)

    # causal masks: variant m = kb%4; diag block at cols [m*128, m*128+128)
    masks = np.full((P, 4, QC), BIG, np.float32)
    pi, fi = np.meshgrid(np.arange(P), np.arange(P), indexing="ij")
    tri = np.where(pi > fi, NEG, BIG).astype(np.float32)
    for m in range(4):
        masks[:, m, : m * P] = NEG
        masks[:, m, m * P : (m + 1) * P] = tri

    common = {
        "wqkv": np.ascontiguousarray(wqkv_eff).astype(bf),
        "wo_t": np.ascontiguousarray(wo_eff.reshape(DEPTH, E, E)).astype(bf),
        "w1": np.ascontiguousarray(w1_eff).astype(bf),
        "b1_t": b1_t,
        "w2_t": np.ascontiguousarray(w2).astype(bf),
        "hw1": np.ascontiguousarray(hw1_eff).astype(bf),
        "hb1_t": hb1_t,
        "hw2": np.ascontiguousarray(np.pad(hw2, ((0, 0), (0, 1)))).astype(bf),
        "alibi_qe": alibi_qe,
        "kones": np.ones((EC, N)).astype(bf),
        "alibi_b": alibi_b,
        "masks": masks,
        "ident": np.eye(P).astype(bf),
    }
    assert np.all(bqkv_eff == 0) and np.all(bo == 0), "nonzero qkv/out-proj biases not wired"
    return common


def kernel(**inputs):
    from concourse.bass_utils import run_bass_kernel_spmd

    common = _host_prep(inputs)
    if "nc" not in _CACHE:
        _CACHE["nc"] = build_program()
    nc = _CACHE["nc"]

    x = np.asarray(inputs["x"], np.float32)
    in_maps = [dict(common, x=np.ascontiguousarray(x[c])) for c in range(B)]
    trace = bool(int(os.environ.get("KERNEL_TRACE", "0")))
    res = run_bass_kernel_spmd(nc, in_maps, list(range(B)), trace=trace,
                               tmpdir=os.environ.get("KERNEL_TRACE_DIR"))
    if trace:
        print(f"HW exec time: {res.exec_time_ns} ns")
    return np.stack([res.results[c]["out"] for c in range(B)]).astype(np.float32)


# revision 43
# speedup vs baseline: 1.0131x; 1.0131x over previous
"""Trainium2 Bass kernel for a 6-layer GPT-style transformer (ALiBi + causal),
data-parallel over batch across 8 NeuronCores (1 sequence per core).

v2 layout strategy per core (N=1024 tokens, E=768, H=12, DH=64):
  - Residual x: token-major SBUF [128, 8, 768] fp32 (partition = token % 128)
  - LN on token-major tiles; normalized y cast to bf16, PE-transposed to
    feature-major yT [128, 6, 1024] bf16.
  - Every GEMM uses bf16 stationary+moving (fp32 PSUM accumulation):
    cheap LDWEIGHTS and 1 cycle/row streaming.
  - q/k projections: per-head chains -> psum [64, 512], copied into
    persistent 65-partition tiles q65/k65 [65, 12, 1024] bf16 whose row 64
    holds constants written once: q row 64 = -slope_h*i/SCALE (safe-exp
    shift per query), k row 64 = 1.0.  The score matmul then computes
    k65^T @ q65 with a 65-deep contraction: scores + per-query shift in a
    SINGLE matmul per 512-wide chunk (no rank-1 bias matmul).
  - ALiBi "+slope*j" via per-partition ACT bias at the exp; causal mask via
    DVE min with 4 precomputed [128,512] triangle tiles (one per kb%4).
  - attn*v and the softmax denominator are two chains into one [128,512]
    psum pair: even head -> partitions 0:64, odd head -> 64:128 (PE
    tile_position), so normalized oT lands pre-paired [128, 6, 1024] and
    out-proj contracts 128-deep in 6 steps.
  - MLP: fc1 feature-major (gelu fused via ACT), fc2 token-major, both bf16.
"""

import math
import os
import sys

import numpy as np

sys.path.insert(0, "/opt/trn_rl_repo")

import concourse.bass as bass  # noqa: E402
import concourse.mybir as mybir  # noqa: E402
import concourse.tile as tile  # noqa: E402
from concourse import bacc  # noqa: E402

P = 128
B, N, E, H, DEPTH, A = 8, 1024, 768, 12, 6, 7
DH = E // H  # 64
F = 4 * E  # 3072
NT = N // P  # 8 token tiles
EC = E // P  # 6 E chunks
FC = F // P  # 24 F chunks
QC = 512  # query chunk for scores / o
SCALE = DH ** -0.5  # 0.125
EPS = 1e-6
NEG = -1.0e30
BIG = 3.0e38

f32 = mybir.dt.float32
bf16 = mybir.dt.bfloat16
AF = mybir.ActivationFunctionType
ALU = mybir.AluOpType

# Heads are processed in packed pairs (slotA, slotB). Slot A (partitions
# 0..64) carries the exact per-query safe-exp shift via a 65th contraction
# row; slot B heads have small ALiBi slopes, so a constant re-centered exp
# bias slope*(j-512) is overflow-safe and softmax-invariant — no shift
# needed. PERM packs the 5 big-slope heads (+h2) into slot A.
PERM = [0, 3, 1, 4, 8, 5, 9, 6, 10, 7, 2, 11]
J0 = 512  # re-center offset for slot-B exp bias

# att strip kb stores query cols [qstart(kb), N), qstart = (kb//4)*512
ATT_OFF = [0, 1024, 2048, 3072, 4096, 4608, 5120, 5632]
ATT_TOT = 6144


def _slopes(n):
    def p2(n):
        start = 2 ** (-(2 ** -(math.log2(n) - 3)))
        return [start * start**i for i in range(n)]

    if math.log2(n).is_integer():
        return p2(n)
    c = 2 ** math.floor(math.log2(n))
    return p2(c) + _slopes(2 * c)[0::2][: n - c]


def build_program(sim_gelu=False, finalize=True, depth=DEPTH,
                  skip_attn=False, skip_mlp=False):
    nc = bacc.Bacc()

    dp = nc.declare_dram_parameter
    x_d = dp("x", [N, E], f32, isOutput=False)
    wqkv_d = dp("wqkv", [DEPTH, E, 3 * E], bf16, isOutput=False)
    wo_d = dp("wo_t", [DEPTH, E, E], bf16, isOutput=False)
    w1_d = dp("w1", [DEPTH, E, F], bf16, isOutput=False)
    b1_d = dp("b1_t", [DEPTH, P, FC], f32, isOutput=False)
    w2_d = dp("w2_t", [DEPTH, F, E], bf16, isOutput=False)
    hw1_d = dp("hw1", [E, E], bf16, isOutput=False)
    hb1_d = dp("hb1_t", [P, EC], f32, isOutput=False)
    hw2_d = dp("hw2", [E, 8], bf16, isOutput=False)  # A=7 padded to 8
    aqe_d = dp("alibi_qe", [EC, N], bf16, isOutput=False)  # slot-A shift rows
    ko_d = dp("kones", [EC, N], bf16, isOutput=False)  # ones row for slot-A k65
    ab_d = dp("alibi_b", [P, H * NT], f32, isOutput=False)  # slope_h*(128kb+p)
    mk_d = dp("masks", [P, 4, QC], f32, isOutput=False)  # causal masks by kb%4
    id_d = dp("ident", [P, P], bf16, isOutput=False)
    out_d = dp("out", [N, A], f32, isOutput=True)

    with tile.TileContext(nc) as tc:
        with tc.tile_pool(name="persist", bufs=1) as pp:
            x_sb = pp.tile([P, NT, E], f32)
            yT = pp.tile([P, EC, N], bf16)
            oT = pp.tile([P, EC, N], bf16)
            ones64 = pp.tile([P, DH], bf16)
            ab_sb = pp.tile([P, H * NT], f32)
            mk_sb = pp.tile([P, 4, QC], f32)
            id_sb = pp.tile([P, P], bf16)

            nc.sync.dma_start(x_sb[:], x_d.rearrange("(t p) e -> p t e", p=P))
            nc.sync.dma_start(ab_sb[:], ab_d[:])
            nc.sync.dma_start(mk_sb[:], mk_d[:])
            nc.sync.dma_start(id_sb[:], id_d[:])
            nc.vector.memset(ones64[:], 1.0)

            czero = pp.tile([P, 1], f32)
            ceps = pp.tile([P, 1], f32)
            nc.vector.memset(czero[:], 0.0)
            nc.vector.memset(ceps[:], EPS)
            nc.const_aps.aps[(f32, 0.0)] = czero[:]
            nc.const_aps.aps[(f32, EPS)] = ceps[:]

            for layer in range(depth):
                _layernorm_to_yT(nc, tc, x_sb, yT, id_sb)
                if not skip_attn:
                    _attn_block(nc, tc, layer, x_sb, yT, oT, ones64,
                                ab_sb, mk_sb, wqkv_d, wo_d, aqe_d, ko_d)
                    _layernorm_to_yT(nc, tc, x_sb, yT, id_sb)
                if not skip_mlp:
                    _mlp_block(nc, tc, layer, x_sb, yT, w1_d, b1_d, w2_d,
                               sim_gelu)

            _head(nc, tc, x_sb, yT, id_sb, hw1_d, hb1_d, hw2_d, out_d)

    if finalize:
        nc.finalize()
    return nc


def _layernorm_to_yT(nc, tc, x_sb, yT, id_sb):
    """token-major LN over x_sb -> feature-major bf16 yT (PE transpose)."""
    from contextlib import ExitStack

    with ExitStack() as ctx:
        lp = ctx.enter_context(tc.tile_pool(name="ln", bufs=2))
        sp = ctx.enter_context(tc.tile_pool(name="lns", bufs=1))
        ps = ctx.enter_context(tc.tile_pool(name="lnp", bufs=2, space="PSUM"))

        stats = sp.tile([P, NT, 2], f32)
        rstd = sp.tile([P, NT], f32)
        nmr = sp.tile([P, NT], f32)
        for half in range(2):
            t0, t1 = half * 4, half * 4 + 4
            for t in range(t0, t1):
                st6 = lp.tile([P, 2, 6], f32, tag="st6")
                nc.vector.bn_stats(st6[:, 0, :], x_sb[:, t, 0:384])
                nc.vector.bn_stats(st6[:, 1, :], x_sb[:, t, 384:768])
                nc.vector.bn_aggr(stats[:, t, :], st6[:])
            sl = slice(t0, t1)
            nc.scalar.activation(stats[:, sl, 1], stats[:, sl, 1], AF.Ln, bias=EPS)
            nc.scalar.activation(rstd[:, sl], stats[:, sl, 1], AF.Exp, scale=-0.5)
            nc.vector.tensor_tensor(nmr[:, sl], stats[:, sl, 0], rstd[:, sl], ALU.mult)
            nc.vector.tensor_scalar_mul(nmr[:, sl], nmr[:, sl], -1.0)
            for t in range(t0, t1):
                y_t = lp.tile([P, E], bf16, tag="y")
                nc.scalar.activation(
                    y_t[:], x_sb[:, t, :], AF.Identity,
                    bias=nmr[:, t : t + 1], scale=rstd[:, t : t + 1],
                )
                tp = ps.tile([P, E], bf16, tag="tp")
                for c in range(EC):
                    nc.tensor.transpose(tp[:, c * P : (c + 1) * P], y_t[:, c * P : (c + 1) * P], id_sb[:])
                nc.vector.tensor_copy(
                    yT[:, :, t * P : (t + 1) * P],
                    tp[:].rearrange("p (c q) -> p c q", q=P),
                )


def _attn_block(nc, tc, layer, x_sb, yT, oT, ones64,
                ab_sb, mk_sb, wqkv_d, wo_d, aqe_d, ko_d):
    from contextlib import ExitStack

    with ExitStack() as octx:
        op = octx.enter_context(tc.tile_pool(name="qk5", bufs=1))
        # slot-A heads: 65-row tiles at partitions 0..64 (row 64 = constants)
        q65 = op.tile([DH + 1, EC, N], bf16)
        k65 = op.tile([DH + 1, EC, N], bf16)
        # slot-B heads: rows 64..127 (constant re-centered bias, no shift)
        qo = op.tile([P, EC, N], bf16)
        ko = op.tile([P, EC, N], bf16)
        vhat = op.tile([P, NT, E], bf16)
        nc.sync.dma_start(q65[DH : DH + 1, :, :], aqe_d.rearrange("(o g) n -> o g n", o=1))
        nc.sync.dma_start(k65[DH : DH + 1, :, :], ko_d.rearrange("(o g) n -> o g n", o=1))
        _attn_inner(nc, tc, layer, x_sb, yT, q65, k65, qo, ko, oT, vhat,
                    ones64, ab_sb, mk_sb, wqkv_d, wo_d)


def _attn_inner(nc, tc, layer, x_sb, yT, q65, k65, qo, ko, oT, vhat,
                ones64, ab_sb, mk_sb, wqkv_d, wo_d):
    from contextlib import ExitStack

    # ---------- v = y @ wv  (token-major) ----------
    with ExitStack() as ctx:
        wp = ctx.enter_context(tc.tile_pool(name="vw", bufs=2))
        ps = ctx.enter_context(tc.tile_pool(name="vp", bufs=2, space="PSUM"))
        vw0 = wp.tile([P, EC, 512], bf16, tag="vw0")
        vw1 = wp.tile([P, EC, 256], bf16, tag="vw1")
        nc.sync.dma_start(
            vw0[:], wqkv_d[layer, :, 2 * E : 2 * E + 512].rearrange("(o p) c -> p o c", p=P)
        )
        nc.sync.dma_start(
            vw1[:], wqkv_d[layer, :, 2 * E + 512 :].rearrange("(o p) c -> p o c", p=P)
        )
        for t in range(NT):
            pv0 = ps.tile([P, 512], f32, tag="pv0")
            pv1 = ps.tile([P, 256], f32, tag="pv1")
            for ec in range(EC):
                nc.tensor.matmul(
                    pv0[:], yT[:, ec, t * P : (t + 1) * P], vw0[:, ec, :],
                    start=(ec == 0), stop=(ec == EC - 1),
                )
            for ec in range(EC):
                nc.tensor.matmul(
                    pv1[:], yT[:, ec, t * P : (t + 1) * P], vw1[:, ec, :],
                    start=(ec == 0), stop=(ec == EC - 1),
                )
            nc.vector.tensor_copy(vhat[:, t, 0:512], pv0[:])
            nc.vector.tensor_copy(vhat[:, t, 512:768], pv1[:])

    # ---------- q/k projections: one pair of heads per [128,512] psum ----------
    with ExitStack() as ctx:
        qkw = ctx.enter_context(tc.tile_pool(name="qkw", bufs=3))
        pqk = ctx.enter_context(tc.tile_pool(name="pqk", bufs=3, space="PSUM"))
        for g in range(H // 2):
            qw = qkw.tile([P, EC, P], bf16, tag="qw")
            kw = qkw.tile([P, EC, P], bf16, tag="kw")
            nc.sync.dma_start(
                qw[:], wqkv_d[layer, :, g * P : (g + 1) * P].rearrange("(o p) c -> p o c", p=P)
            )
            nc.sync.dma_start(
                kw[:], wqkv_d[layer, :, E + g * P : E + (g + 1) * P].rearrange("(o p) c -> p o c", p=P)
            )
            for tb in range(2):
                tsl = slice(tb * 512, (tb + 1) * 512)
                pq = pqk.tile([P, 512], f32, tag="pqk")
                pk = pqk.tile([P, 512], f32, tag="pqk")
                for ec in range(EC):
                    nc.tensor.matmul(
                        pq[:], qw[:, ec, :], yT[:, ec, tsl],
                        start=(ec == 0), stop=(ec == EC - 1),
                    )
                for ec in range(EC):
                    nc.tensor.matmul(
                        pk[:], kw[:, ec, :], yT[:, ec, tsl],
                        start=(ec == 0), stop=(ec == EC - 1),
                    )
                nc.scalar.copy(q65[0:DH, g, tsl], pq[0:DH, :])
                nc.scalar.copy(qo[DH:P, g, tsl], pq[DH:P, :])
                nc.vector.tensor_copy(k65[0:DH, g, tsl], pk[0:DH, :])
                nc.vector.tensor_copy(ko[DH:P, g, tsl], pk[DH:P, :])

    # ---------- scores/exp + o/r ----------
    with ExitStack() as ctx:
        att = ctx.enter_context(tc.tile_pool(name="att", bufs=3))
        rcp = ctx.enter_context(tc.tile_pool(name="rcp", bufs=2))
        pst = ctx.enter_context(tc.tile_pool(name="pst", bufs=4, space="PSUM"))
        pov = ctx.enter_context(tc.tile_pool(name="pov", bufs=2, space="PSUM"))
        prv = ctx.enter_context(tc.tile_pool(name="prv", bufs=2, space="PSUM"))

        att_h = {}

        def emit_or(g):
            # o / r for pair g (issued one pair late so its exps are drained)
            for qb in range(2):
                po = pov.tile([P, QC], f32, tag="po")
                pr = prv.tile([P, QC], f32, tag="pr")
                kbs = list(range(4 * (qb + 1)))
                for hh in range(2):
                    hcur = 2 * g + hh
                    a = att_h[hcur]
                    osl = slice(hh * DH, (hh + 1) * DH)
                    for kb in kbs:
                        off = ATT_OFF[kb] + qb * 512 - (kb // 4) * 512
                        nc.tensor.matmul(
                            po[osl, :], vhat[:, kb, hcur * DH : (hcur + 1) * DH],
                            a[:, off : off + QC],
                            start=(kb == 0), stop=(kb == kbs[-1]),
                        )
                    for kb in kbs:
                        off = ATT_OFF[kb] + qb * 512 - (kb // 4) * 512
                        nc.tensor.matmul(
                            pr[osl, :], ones64[:], a[:, off : off + QC],
                            start=(kb == 0), stop=(kb == kbs[-1]),
                        )
                rec = rcp.tile([P, QC], f32, tag="rec")
                nc.vector.reciprocal_approx_fast(rec[:], pr[:])
                nc.vector.tensor_tensor(
                    oT[:, g, qb * QC : (qb + 1) * QC], po[:], rec[:], ALU.mult
                )

        for h in range(H):
            g = h // 2
            # -- scores + exp for head h --
            at = att.tile([P, ATT_TOT], bf16, tag="at")
            att_h[h] = at
            for kb in range(NT):
                qstart = (kb // 4) * 512
                m4 = kb % 4
                for ci, q0 in enumerate(range(qstart, N, QC)):
                    st = pst.tile([P, QC], f32, tag="st")
                    if h % 2 == 0:
                        nc.tensor.matmul(
                            st[:],
                            k65[:, g, kb * P : (kb + 1) * P],
                            q65[:, g, q0 : q0 + QC],
                            start=True, stop=True,
                        )
                    else:
                        nc.tensor.matmul(
                            st[:],
                            ko[DH:P, g, kb * P : (kb + 1) * P],
                            qo[DH:P, g, q0 : q0 + QC],
                            start=True, stop=True,
                        )
                    if ci == 0:
                        w = (m4 + 1) * P
                        nc.vector.tensor_tensor(
                            st[:, 0:w], st[:, 0:w], mk_sb[:, m4, 0:w], ALU.min
                        )
                    nc.scalar.activation(
                        at[:, ATT_OFF[kb] + q0 - qstart : ATT_OFF[kb] + q0 - qstart + QC],
                        st[:], AF.Exp,
                        bias=ab_sb[:, h * NT + kb : h * NT + kb + 1], scale=SCALE,
                    )
            if h % 2 == 1:
                emit_or(g)

    # ---------- out-proj + residual ----------
    with ExitStack() as ctx:
        wp = ctx.enter_context(tc.tile_pool(name="wop", bufs=1))
        ps = ctx.enter_context(tc.tile_pool(name="pop", bufs=2, space="PSUM"))
        wo_sb = wp.tile([P, EC, E], bf16)
        nc.sync.dma_start(wo_sb[:], wo_d[layer].rearrange("(o p) c -> p o c", p=P))
        for t in range(NT):
            po0 = ps.tile([P, 512], f32, tag="po0")
            po1 = ps.tile([P, 256], f32, tag="po1")
            for g in range(EC):
                nc.tensor.matmul(
                    po0[:], oT[:, g, t * P : (t + 1) * P], wo_sb[:, g, 0:512],
                    start=(g == 0), stop=(g == EC - 1),
                )
            for g in range(EC):
                nc.tensor.matmul(
                    po1[:], oT[:, g, t * P : (t + 1) * P], wo_sb[:, g, 512:768],
                    start=(g == 0), stop=(g == EC - 1),
                )
            nc.vector.tensor_tensor(x_sb[:, t, 0:512], po0[:], x_sb[:, t, 0:512], ALU.add)
            nc.vector.tensor_tensor(x_sb[:, t, 512:768], po1[:], x_sb[:, t, 512:768], ALU.add)


def _mlp_block(nc, tc, layer, x_sb, yT, w1_d, b1_d, w2_d, sim_gelu):
    from contextlib import ExitStack

    with ExitStack() as ctx:
        wp = ctx.enter_context(tc.tile_pool(name="mw", bufs=4))
        hp = ctx.enter_context(tc.tile_pool(name="hT", bufs=1))
        bp = ctx.enter_context(tc.tile_pool(name="b1", bufs=1))
        ps1 = ctx.enter_context(tc.tile_pool(name="pf1", bufs=2, space="PSUM"))
        ps2 = ctx.enter_context(tc.tile_pool(name="pf2", bufs=2, space="PSUM"))

        hT = hp.tile([P, FC, N], bf16)
        b1 = bp.tile([P, FC], f32)
        nc.sync.dma_start(b1[:], b1_d[layer])
        for fc in range(FC):
            w1c = wp.tile([P, EC, P], bf16, tag="w1c")
            nc.sync.dma_start(
                w1c[:], w1_d[layer, :, fc * P : (fc + 1) * P].rearrange("(o p) c -> p o c", p=P)
            )
            for tb in range(2):
                pf = ps1.tile([P, 512], f32, tag="pf")
                for ec in range(EC):
                    nc.tensor.matmul(
                        pf[:], w1c[:, ec, :], yT[:, ec, tb * 512 : (tb + 1) * 512],
                        start=(ec == 0), stop=(ec == EC - 1),
                    )
                dst = hT[:, fc, tb * 512 : (tb + 1) * 512]
                if not sim_gelu:
                    nc.scalar.activation(
                        dst, pf[:], AF.Gelu_apprx_tanh, bias=b1[:, fc : fc + 1]
                    )
                else:
                    u = wp.tile([P, 512], f32, tag="sg_u")
                    s = wp.tile([P, 512], f32, tag="sg_s")
                    th = wp.tile([P, 512], f32, tag="sg_t")
                    nc.scalar.activation(u[:], pf[:], AF.Identity, bias=b1[:, fc : fc + 1])
                    nc.vector.tensor_tensor(s[:], u[:], u[:], ALU.mult)
                    nc.vector.tensor_scalar(s[:], s[:], 0.044715, 1.0, ALU.mult, ALU.add)
                    nc.vector.tensor_tensor(s[:], s[:], u[:], ALU.mult)
                    nc.scalar.activation(th[:], s[:], AF.Tanh, scale=0.7978845608028654)
                    nc.vector.tensor_tensor(th[:], th[:], u[:], ALU.mult)
                    nc.vector.tensor_tensor(th[:], th[:], u[:], ALU.add)
                    nc.vector.tensor_scalar_mul(dst, th[:], 0.5)
        w2_sb = hp.tile([P, FC, E], bf16)
        nc.sync.dma_start(w2_sb[:], w2_d[layer].rearrange("(o p) c -> p o c", p=P))
        for t in range(NT):
            pm0 = ps2.tile([P, 512], f32, tag="pm0")
            pm1 = ps2.tile([P, 256], f32, tag="pm1")
            for fc in range(FC):
                nc.tensor.matmul(
                    pm0[:], hT[:, fc, t * P : (t + 1) * P], w2_sb[:, fc, 0:512],
                    start=(fc == 0), stop=(fc == FC - 1),
                )
            for fc in range(FC):
                nc.tensor.matmul(
                    pm1[:], hT[:, fc, t * P : (t + 1) * P], w2_sb[:, fc, 512:768],
                    start=(fc == 0), stop=(fc == FC - 1),
                )
            nc.vector.tensor_tensor(x_sb[:, t, 0:512], pm0[:], x_sb[:, t, 0:512], ALU.add)
            nc.vector.tensor_tensor(x_sb[:, t, 512:768], pm1[:], x_sb[:, t, 512:768], ALU.add)


def _head(nc, tc, x_sb, yT, id_sb, hw1_d, hb1_d, hw2_d, out_d):
    from contextlib import ExitStack

    _layernorm_to_yT(nc, tc, x_sb, yT, id_sb)
    with ExitStack() as ctx:
        wp = ctx.enter_context(tc.tile_pool(name="hw", bufs=1))
        hp = ctx.enter_context(tc.tile_pool(name="h1", bufs=1))
        op = ctx.enter_context(tc.tile_pool(name="ot", bufs=1))
        ps = ctx.enter_context(tc.tile_pool(name="ph", bufs=2, space="PSUM"))

        hb1 = wp.tile([P, EC], f32)
        nc.sync.dma_start(hb1[:], hb1_d[:])
        hw2 = wp.tile([P, EC, 8], bf16)
        nc.sync.dma_start(hw2[:], hw2_d.rearrange("(o p) a -> p o a", p=P))
        h1T = hp.tile([P, EC, N], bf16)
        for oc in range(EC):
            w1c = wp.tile([P, EC, P], bf16, tag="hw1c", bufs=2)
            nc.sync.dma_start(
                w1c[:], hw1_d[:, oc * P : (oc + 1) * P].rearrange("(o p) c -> p o c", p=P)
            )
            for tb in range(2):
                pf = ps.tile([P, 512], f32, tag="pf")
                for ec in range(EC):
                    nc.tensor.matmul(
                        pf[:], w1c[:, ec, :], yT[:, ec, tb * 512 : (tb + 1) * 512],
                        start=(ec == 0), stop=(ec == EC - 1),
                    )
                nc.scalar.activation(
                    h1T[:, oc, tb * 512 : (tb + 1) * 512], pf[:], AF.Relu,
                    bias=hb1[:, oc : oc + 1],
                )
        out_sb = op.tile([P, NT, A], f32)
        for t in range(NT):
            pa = ps.tile([P, 8], f32, tag="pa")
            for ec in range(EC):
                nc.tensor.matmul(
                    pa[:], h1T[:, ec, t * P : (t + 1) * P], hw2[:, ec, :],
                    start=(ec == 0), stop=(ec == EC - 1),
                )
            nc.vector.tensor_copy(out_sb[:, t, :], pa[:, 0:A])
        nc.sync.dma_start(out_d.rearrange("(t p) a -> p t a", p=P), out_sb[:])


# ---------------------------------------------------------------- host side

_CACHE = {}


def ml_bf16():
    import ml_dtypes

    return ml_dtypes.bfloat16


def _host_prep(inputs):
    bf = ml_bf16()
    slopes = np.asarray(_slopes(H), np.float32)
    i_idx = np.arange(N, dtype=np.float32)

    ln1_s, ln1_b = np.asarray(inputs["ln1_scale"]), np.asarray(inputs["ln1_bias"])
    ln2_s, ln2_b = np.asarray(inputs["ln2_scale"]), np.asarray(inputs["ln2_bias"])
    lnf_s, lnf_b = np.asarray(inputs["lnf_scale"]), np.asarray(inputs["lnf_bias"])
    wqkv, bqkv = np.asarray(inputs["wqkv"]), np.asarray(inputs["bqkv"])
    wo, bo = np.asarray(inputs["wo"]), np.asarray(inputs["bo"])
    w1, w2 = np.asarray(inputs["w1"]), np.asarray(inputs["w2"])
    hw1, hb1 = np.asarray(inputs["head_w1"]), np.asarray(inputs["head_b1"])
    hw2 = np.asarray(inputs["head_w2"])

    # fold LN affine into following matmuls (exact algebra)
    wqkv_eff = ln1_s[:, :, None] * wqkv
    bqkv_eff = bqkv + np.einsum("le,lec->lc", ln1_b, wqkv)
    w1_eff = ln2_s[:, :, None] * w1
    b1_eff = np.einsum("le,lef->lf", ln2_b, w1)
    hw1_eff = lnf_s[:, None] * hw1
    hb1_eff = hb1 + lnf_b @ hw1

    # pack heads in PERM order: q/k/v weight columns and wo rows
    hidx = np.concatenate([np.arange(p * DH, (p + 1) * DH) for p in PERM])
    wqkv_eff = np.concatenate(
        [wqkv_eff[:, :, hidx], wqkv_eff[:, :, E + hidx], wqkv_eff[:, :, 2 * E + hidx]],
        axis=2,
    )
    wo_eff = wo[:, hidx, :]

    b1_t = np.ascontiguousarray(b1_eff.reshape(DEPTH, FC, P).transpose(0, 2, 1))
    hb1_t = np.ascontiguousarray(hb1_eff.reshape(EC, P).T)

    aq_row = ((-slopes[:, None] / SCALE) * i_idx[None, :]).astype(bf)  # [H,N]
    alibi_qe = np.ascontiguousarray(aq_row[PERM[0::2]])  # slot-A heads
    alibi_b = np.zeros((P, H * NT), np.float32)
    p_idx = np.arange(P, dtype=np.float32)
    for s in range(H):
        t = PERM[s]
        j0 = 0 if s % 2 == 0 else J0  # slot-B: overflow-safe re-centering
        for kb in range(NT):
            alibi_b[:, s * NT + kb] = slopes[t] * (P * kb + p_idx - j0)

    # causal masks: variant m = kb%4; diag block at cols [m*128, m*128+128)
    masks = np.full((P, 4, QC), BIG, np.float32)
    pi, fi = np.meshgrid(np.arange(P), np.arange(P), indexing="ij")
    tri = np.where(pi > fi, NEG, BIG).astype(np.float32)
    for m in range(4):
        masks[:, m, : m * P] = NEG
        masks[:, m, m * P : (m + 1) * P] = tri

    common = {
        "wqkv": np.ascontiguousarray(wqkv_eff).astype(bf),
        "wo_t": np.ascontiguousarray(wo_eff.reshape(DEPTH, E, E)).astype(bf),
        "w1": np.ascontiguousarray(w1_eff).astype(bf),
        "b1_t": b1_t,
        "w2_t": np.ascontiguousarray(w2).astype(bf),
        "hw1": np.ascontiguousarray(hw1_eff).astype(bf),
        "hb1_t": hb1_t,
        "hw2": np.ascontiguousarray(np.pad(hw2, ((0, 0), (0, 1)))).astype(bf),
        "alibi_qe": alibi_qe,
        "kones": np.ones((EC, N)).astype(bf),
        "alibi_b": alibi_b,
        "masks": masks,
        "ident": np.eye(P).astype(bf),
    }
    assert np.all(bqkv_eff == 0) and np.all(bo == 0), "nonzero qkv/out-proj biases not wired"
    return common


def kernel(**inputs):
    from concourse.bass_utils import run_bass_kernel_spmd

    common = _host_prep(inputs)
    if "nc" not in _CACHE:
        _CACHE["nc"] = build_program()
    nc = _CACHE["nc"]

    x = np.asarray(inputs["x"], np.float32)
    in_maps = [dict(common, x=np.ascontiguousarray(x[c])) for c in range(B)]
    trace = bool(int(os.environ.get("KERNEL_TRACE", "0")))
    res = run_bass_kernel_spmd(nc, in_maps, list(range(B)), trace=trace,
                               tmpdir=os.environ.get("KERNEL_TRACE_DIR"))
    if trace:
        print(f"HW exec time: {res.exec_time_ns} ns")
    return np.stack([res.results[c]["out"] for c in range(B)]).astype(np.float32)
